# revision 1
# baseline (speedup 1.0000x reference)
"""nn_BlockCirculantLinear on 8 Trainium2 cores — polyphase factorization.

Each 512-circulant block is decomposed by polyphase (Q=4) into a 4x4 grid
of 128-circulants, all diagonalized by one shared real-DFT_128.  The whole
4096x4096 operator then factors as
  fwd : per comp (i,q) of 32:  X_c = Fe128 @ x_c        (1 matmul, shared lhsT)
  mid : per coord group g of 32: O_g = MID_g @ D_g      (1 matmul each)
  inv : per comp (o,r) of 32:  y_c' = Fi128 @ Z_c'      (1 matmul, shared lhsT)
i.e. 96 matmuls of [K=128,M=128,N=chunk] per chunk vs 512 for the dense-tile
formulation.  The comp<->freq corner turns (T1/T2) are single whole-tile
SBUF->SBUF DMAs with 4-D strided APs (contiguous per-row runs, all 128
partitions busy on both sides).  All data bf16, PSUM fp32.

Data-parallel over rows: 16384 rows split 8 ways; constants replicated.
Host folds sign_flip, pre-permutes x into polyphase component order,
un-permutes y and adds bias (free input/output marshalling).
"""
import os
from contextlib import ExitStack

import numpy as np
import ml_dtypes

import concourse.mybir as mybir
import concourse.bacc as bacc
import concourse.tile as tile
from concourse.bass_utils import run_bass_kernel_spmd

N_CORES = 8
ROWS = 16384
RPC = ROWS // N_CORES      # 2048 rows per core
F = 4096
P = 512
Q = 4
PP = P // Q                # 128 polyphase transform length
NB = 8                     # circulant blocks per side
NC = NB * Q                # 32 comps per side
CHUNK = 256                # rows per pipelined chunk (= matmul free dim)
_NC_CACHE = {}

DT = mybir.dt.bfloat16
DTO = mybir.dt.float32


def build_transforms_g(lam_r, lam_i, PP, dtype=np.float64):
    """Real-folded DFT transforms for circulants of size PP.
    lam_*: (CO, CI, PP) spectra. Returns Fe [c,s], Fi [t,c], M [CO,CI,c,c]."""
    s = np.arange(PP)
    f = np.arange(1, PP // 2)
    theta = 2 * np.pi * np.outer(f, s) / PP

    Fe = np.zeros((PP, PP), dtype)
    Fe[0, :] = 1.0
    Fe[1, :] = (-1.0) ** s
    Fe[2::2, :] = np.cos(theta)
    Fe[3::2, :] = -np.sin(theta)

    Fi = np.zeros((PP, PP), dtype)
    Fi[:, 0] = 1.0 / PP
    Fi[:, 1] = ((-1.0) ** s) / PP
    Fi[:, 2::2] = np.cos(theta).T / PP
    Fi[:, 3::2] = -np.sin(theta).T / PP

    CO, CI = lam_r.shape[:2]
    M = np.zeros((CO, CI, PP, PP), dtype)
    M[:, :, 0, 0] = lam_r[:, :, 0]
    M[:, :, 1, 1] = lam_r[:, :, PP // 2]
    l1r = lam_r[:, :, 1:PP // 2]; l1i = lam_i[:, :, 1:PP // 2]
    l2r = lam_r[:, :, :PP // 2:-1]; l2i = lam_i[:, :, :PP // 2:-1]
    ce = np.arange(2, PP, 2); co = ce + 1
    M[:, :, ce, ce] = l1r + l2r
    M[:, :, ce, co] = l2i - l1i
    M[:, :, co, ce] = l1i - l2i
    M[:, :, co, co] = l1r + l2r
    return Fe, Fi, M


def poly_spectra(spectral_real, spectral_imag):
    """Lam512 (8,8,512) -> polyphase 128-spectra lam[(4o+r),(4i+q),f]."""
    Lam = spectral_real.astype(np.complex128) + 1j * spectral_imag.astype(np.complex128)
    w = np.fft.ifft(Lam, axis=-1).real                    # (8,8,512) symbols
    m = np.arange(PP)
    r = np.arange(Q)[:, None, None]
    q = np.arange(Q)[None, :, None]
    idx = (Q * m[None, None, :] + r - q) % P              # (r, q, m)
    sym = w[:, :, idx]                                    # (o, i, r, q, m)
    lam = np.fft.fft(sym, axis=-1)                        # (o, i, r, q, f)
    lam = lam.transpose(0, 2, 1, 3, 4).reshape(NC, NC, PP)
    return lam


def host_transforms(spectral_real, spectral_imag):
    lam = poly_spectra(spectral_real, spectral_imag)
    Fe, Fi, M = build_transforms_g(lam.real, lam.imag, PP)
    # SBUF partition order p = 32j+g <-> natural coord 4g+j (so the corner
    # turn DMAs walk consecutive partitions); PI maps p -> coord.
    PI = np.array([4 * g + j for j in range(Q) for g in range(NC)])
    fwdT = np.ascontiguousarray(Fe[PI, :].T)              # lhsT [s, p]
    invT = np.ascontiguousarray(Fi[:, PI].T)              # lhsT [p, t]
    # mid lhsT per coord group g: [k=(32j+c), m=(32j'+c')]
    midT = np.empty((NC, PP, PP))
    for g in range(NC):
        blk = M[:, :, 4 * g:4 * g + 4, 4 * g:4 * g + 4]   # (c', c, j', j)
        midT[g] = blk.transpose(3, 1, 2, 0).reshape(PP, PP)
    return fwdT, invT, midT


def _bf16(a):
    return np.asarray(a, np.float32).astype(ml_dtypes.bfloat16)


def build_nc(repeat: int = 1, rpc: int = RPC, chunk: int = CHUNK,
             variant: str = "pipe"):
    key = (rpc, chunk, repeat, variant)
    if key in _NC_CACHE:
        return _NC_CACHE[key]
    nc = bacc.Bacc("TRN2", target_bir_lowering=False, debug=False,
                   num_devices=N_CORES)
    n_chunks = rpc // chunk
    # chunk-contiguous DRAM layouts: 16 KiB contiguous per partition per DMA
    xT = nc.dram_tensor("xT", [n_chunks, PP, NC, chunk], DT,
                        kind="ExternalInput")
    fwdT = nc.dram_tensor("fwdT", [PP, PP], DT, kind="ExternalInput")
    invT = nc.dram_tensor("invT", [PP, PP], DT, kind="ExternalInput")
    midT = nc.dram_tensor("midT", [NC, PP, PP], DT, kind="ExternalInput")
    yT = nc.dram_tensor("yT", [n_chunks, PP, NC, chunk], DT,
                        kind="ExternalOutput")

    with tile.TileContext(nc) as tc:
        with ExitStack() as ctx:
            const = ctx.enter_context(tc.tile_pool(name="const", bufs=1))
            fwd_sb = const.tile([PP, PP], DT)
            inv_sb = const.tile([PP, PP], DT)
            mid_sb = const.tile([PP, NC * PP], DT)
            nc.sync.dma_start(fwd_sb[:], fwdT[:, :])
            nc.sync.dma_start(inv_sb[:], invT[:, :])
            nc.sync.dma_start(
                mid_sb[:].rearrange("p (g c) -> p g c", g=NC),
                midT[:].rearrange("g p c -> p g c"))

            deep = variant in ("dma_deep", "mm_only", "mm_evict")
            if chunk <= 128:
                bi, bx, bd, bo, bz, by = 4, 3, 3, 3, 3, 4
            else:
                bi, bx, bd, bo, bz, by = 2, 2, 2, 2, 2, 2
            Ipool = ctx.enter_context(tc.tile_pool(
                name="I", bufs=6 if deep else bi))
            Xpool = ctx.enter_context(tc.tile_pool(name="X", bufs=bx))
            Dpool = ctx.enter_context(tc.tile_pool(name="D", bufs=bd))
            Opool = ctx.enter_context(tc.tile_pool(name="O", bufs=bo))
            Zpool = ctx.enter_context(tc.tile_pool(name="Z", bufs=bz))
            Ypool = ctx.enter_context(tc.tile_pool(name="Y", bufs=by))
            psf = ctx.enter_context(tc.tile_pool(name="psf", bufs=2,
                                                 space="PSUM"))
            psm = ctx.enter_context(tc.tile_pool(name="psm", bufs=2,
                                                 space="PSUM"))
            psi = ctx.enter_context(tc.tile_pool(name="psi", bufs=2,
                                                 space="PSUM"))

            do_t = variant in ("full",)
            strided = variant in ("full", "no_transpose")
            do_mm = not variant.startswith("dma_")
            do_evict = variant not in ("mm_only",) and do_mm

            def chunk_body_iso(c):
                # isolation variants: x-DMA + matmuls(rhs=Xin) [+evict] + y-DMA
                Xin = Ipool.tile([PP, NC, chunk], DT, tag="I", name="It")
                if variant == "dma_gp":
                    nc.gpsimd.dma_start(Xin[:], xT[c])
                elif variant == "dma_split":
                    h = NC // 2
                    nc.sync.dma_start(Xin[:, :h, :], xT[c, :, :h, :])
                    nc.scalar.dma_start(Xin[:, h:, :], xT[c, :, h:, :])
                else:
                    nc.sync.dma_start(Xin[:], xT[c])
                Xs = Xpool.tile([PP, NC, chunk], DT, tag="X", name="Xt")
                if do_mm:
                    for i in range(96):
                        pool = (psf, psm, psi)[i % 3]
                        lhs = (fwd_sb, inv_sb)[i % 2]
                        ps = pool.tile([PP, chunk], DTO, tag="p", name="ps")
                        nc.tensor.matmul(ps[:], lhs[:], Xin[:, i % NC, :],
                                         start=True, stop=True)
                        if do_evict:
                            nc.any.tensor_copy(out=Xs[:, i % NC, :], in_=ps[:])
                if variant == "dma_gp":
                    nc.gpsimd.dma_start(yT[c], Xin[:])
                elif variant == "dma_split":
                    h = NC // 2
                    nc.sync.dma_start(yT[c, :, :h, :], Xin[:, :h, :])
                    nc.scalar.dma_start(yT[c, :, h:, :], Xin[:, h:, :])
                else:
                    nc.scalar.dma_start(yT[c], Xin[:])

            def chunk_body(c):
                Xin = Ipool.tile([PP, NC, chunk], DT, tag="I", name="It")
                nc.sync.dma_start(Xin[:], xT[c])
                # spectral, (row, comp)-interleaved free: Xs[32j+g, 32*row+ci]
                xs_dims = [PP, chunk, NC] if strided else [PP, NC, chunk]
                Xs = Xpool.tile(xs_dims, DT, tag="X", name="Xt")
                if do_mm:
                    for ci in range(NC):
                        ps = psf.tile([PP, chunk], DTO, tag="f", name="fps")
                        nc.tensor.matmul(ps[:], fwd_sb[:], Xin[:, ci, :],
                                         start=True, stop=True)
                        dst = Xs[:, :, ci] if strided else Xs[:, ci, :]
                        nc.any.tensor_copy(out=dst, in_=ps[:])
                # T1 corner turn = 32x32 block transpose (j-bands x row-blocks)
                # W[32j+c, 32*row+g] = Xs[32j+g, 32*row+c]
                if do_t:
                    W = Dpool.tile([PP, chunk, NC], DT, tag="D", name="Dt")
                    nc.vector.transpose(W[:], Xs[:])
                else:
                    W = Xs
                # mid: per coord group g, freq-major in/out
                O = Opool.tile(xs_dims, DT, tag="O", name="Ot")
                if do_mm:
                    for g in range(NC):
                        ps = psm.tile([PP, chunk], DTO, tag="m", name="mps")
                        rhs = W[:, :, g] if strided else W[:, g, :]
                        nc.tensor.matmul(ps[:], mid_sb[:, g * PP:(g + 1) * PP],
                                         rhs, start=True, stop=True)
                        dst = O[:, :, g] if strided else O[:, g, :]
                        nc.any.tensor_copy(out=dst, in_=ps[:])
                # T2 corner turn: Z[32j+g, 32*row+c'] = O[32j+c', 32*row+g]
                if do_t:
                    Z = Zpool.tile([PP, chunk, NC], DT, tag="Z", name="Zt")
                    nc.vector.transpose(Z[:], O[:])
                else:
                    Z = O
                Yb = Ypool.tile([PP, NC, chunk], DT, tag="Y", name="Yt")
                if do_mm:
                    for co in range(NC):
                        ps = psi.tile([PP, chunk], DTO, tag="i", name="ips")
                        rhs = Z[:, :, co] if strided else Z[:, co, :]
                        nc.tensor.matmul(ps[:], inv_sb[:], rhs,
                                         start=True, stop=True)
                        nc.any.tensor_copy(out=Yb[:, co, :], in_=ps[:])
                nc.scalar.dma_start(yT[c], Yb[:])

            def body_pipe(_=None):
                # software pipeline: emit stage s of chunk c-s at step c so
                # independent chunks interleave in each engine's program order
                GS = 512 // chunk          # matmuls per PSUM bank
                NG = NC // GS
                live = {}

                def s0(c):
                    Xin = Ipool.tile([PP, NC, chunk], DT, tag="I", name="It")
                    nc.sync.dma_start(Xin[:], xT[c])
                    live[c] = {"Xin": Xin}

                def s1(c):
                    st = live[c]
                    Xs = Xpool.tile([PP, chunk, NC], DT, tag="X", name="Xt")
                    for b in range(NG):
                        ps = psf.tile([PP, 512], DTO, tag="f", name="fps")
                        for k in range(GS):
                            ci = b * GS + k
                            nc.tensor.matmul(
                                ps[:, k * chunk:(k + 1) * chunk], fwd_sb[:],
                                st["Xin"][:, ci, :], start=True, stop=True)
                        nc.any.tensor_copy(
                            out=Xs[:, :, b * GS:(b + 1) * GS]
                            .rearrange("p r c -> p c r"), in_=ps[:])
                    st["Xs"] = Xs

                def s2(c):
                    st = live[c]
                    W = Dpool.tile([PP, chunk, NC], DT, tag="D", name="Dt")
                    nc.vector.transpose(W[:], st["Xs"][:])
                    st["W"] = W

                def s3(c):
                    st = live[c]
                    O = Opool.tile([PP, chunk, NC], DT, tag="O", name="Ot")
                    for b in range(NG):
                        ps = psm.tile([PP, 512], DTO, tag="m", name="mps")
                        for k in range(GS):
                            g = b * GS + k
                            nc.tensor.matmul(
                                ps[:, k * chunk:(k + 1) * chunk],
                                mid_sb[:, g * PP:(g + 1) * PP],
                                st["W"][:, :, g], start=True, stop=True)
                        nc.any.tensor_copy(
                            out=O[:, :, b * GS:(b + 1) * GS]
                            .rearrange("p r c -> p c r"), in_=ps[:])
                    st["O"] = O

                def s4(c):
                    st = live[c]
                    Z = Zpool.tile([PP, chunk, NC], DT, tag="Z", name="Zt")
                    nc.vector.transpose(Z[:], st["O"][:])
                    st["Z"] = Z

                def s5(c):
                    st = live[c]
                    Yb = Ypool.tile([PP, NC, chunk], DT, tag="Y", name="Yt")
                    for b in range(NG):
                        ps = psi.tile([PP, 512], DTO, tag="i", name="ips")
                        for k in range(GS):
                            co = b * GS + k
                            nc.tensor.matmul(
                                ps[:, k * chunk:(k + 1) * chunk], inv_sb[:],
                                st["Z"][:, :, co], start=True, stop=True)
                        nc.any.tensor_copy(
                            out=Yb[:, b * GS:(b + 1) * GS, :], in_=ps[:])
                    st["Yb"] = Yb

                def s6(c):
                    st = live.pop(c)
                    nc.scalar.dma_start(yT[c], st["Yb"][:])

                stages = [s0, s1, s2, s3, s4, s5, s6]
                for t in range(n_chunks + len(stages) - 1):
                    for s in range(len(stages) - 1, -1, -1):
                        cc = t - s
                        if 0 <= cc < n_chunks:
                            stages[s](cc)

            iso = variant in ("mm_only", "mm_evict") or variant.startswith("dma_")

            def body_big(_=None):
                half = n_chunks // 2
                for h in range(2):
                    Xin = Ipool.tile([PP, half * NC * chunk], DT,
                                     tag="I", name="It")
                    nc.sync.dma_start(
                        Xin[:],
                        xT[h * half:(h + 1) * half].rearrange(
                            "n p c r -> p (n c r)"))
                    nc.scalar.dma_start(
                        yT[h * half:(h + 1) * half].rearrange(
                            "n p c r -> p (n c r)"), Xin[:])

            def body(_=None):
                if variant == "dma_big":
                    body_big()
                    return
                if variant == "pipe":
                    body_pipe()
                    return
                for c in range(n_chunks):
                    (chunk_body_iso if iso else chunk_body)(c)

            if repeat == 1:
                body()
            else:
                with tc.For_i(0, repeat, 1) as it:
                    body(it)
    nc.compile()
    _NC_CACHE[key] = nc
    return nc


def make_in_maps(x, spectral_real, spectral_imag, sign_flip):
    fwdT, invT, midT = host_transforms(spectral_real, spectral_imag)
    fwdT = _bf16(fwdT); invT = _bf16(invT); midT = _bf16(midT)
    xs = (x.reshape(-1, F) * sign_flip[None, :].astype(np.float32))
    # polyphase comp permute: comp c=4i+q at partition-feature n <- 512i+4n+q,
    # then chunk-contiguous per core: [n_chunks, PP(n), NC(c), chunk(rows)]
    xp = xs.reshape(-1, NB, PP, Q).transpose(1, 3, 2, 0).reshape(F, -1)
    xp = _bf16(xp)
    n_chunks = RPC // CHUNK
    in_maps = []
    for c in range(N_CORES):
        shard = xp[:, c * RPC:(c + 1) * RPC]               # [4096, RPC]
        xh = shard.reshape(NC, PP, n_chunks, CHUNK).transpose(2, 1, 0, 3)
        in_maps.append({
            "xT": np.ascontiguousarray(xh),
            "fwdT": fwdT, "invT": invT, "midT": midT,
        })
    return in_maps


def kernel(x, spectral_real, spectral_imag, sign_flip, bias):
    x = np.asarray(x, np.float32)
    spectral_real = np.asarray(spectral_real, np.float32)
    spectral_imag = np.asarray(spectral_imag, np.float32)
    sign_flip = np.asarray(sign_flip, np.float32)
    bias = np.asarray(bias, np.float32)
    batch_shape = x.shape[:-1]

    in_maps = make_in_maps(x, spectral_real, spectral_imag, sign_flip)
    nc = build_nc()
    res = run_bass_kernel_spmd(nc, in_maps, list(range(N_CORES)))
    # yh[ch, a, c', r], c'=4o+r2 -> y[ch*CHUNK+r, 512o+4a+r2]
    n_chunks = RPC // CHUNK
    parts = []
    for c in range(N_CORES):
        yh = np.asarray(res.results[c]["yT"], np.float32)
        yp = yh.reshape(n_chunks, PP, NC, CHUNK).transpose(2, 1, 0, 3)
        yp = yp.reshape(F, RPC)                                # [128c'+a, row]
        yp = yp.reshape(NB, Q, PP, RPC).transpose(3, 0, 2, 1).reshape(RPC, F)
        parts.append(yp)
    y = np.concatenate(parts, axis=0)
    y = y + bias[None, :]
    return y.reshape(*batch_shape, F).astype(np.float32)



# revision 3
# speedup vs baseline: 2.5581x; 2.5581x over previous
"""nn_BlockCirculantLinear on 8 Trainium2 cores — polyphase factorization.

Each 512-circulant block is decomposed by polyphase (Q=4) into a 4x4 grid of
128-circulants, all diagonalized by one shared real-DFT_128: the 4096x4096
operator factors into three matmul stages (fwd DFT, per-frequency-group mid
coupling, inverse DFT) with two corner turns between them.

Device pipeline (per 256-row chunk, free dim = 32 comps x 256 rows
interleaved c-minor so PSUM comes out in StreamTranspose-ready layout):
  dma-in -> fwd (16 mm N=512, one stationary FeT) -> E1 evict -> T1 (DVE
  32x32-block StreamTranspose) -> mid (32 mm N=256, per-group MgT) -> E2
  evict (strided dst) -> T2 (DVE ST) -> inv (16 mm N=512, one FiT) -> E3
  evict -> dma-out.
Emission interleaves fwd/mid/inv PSUM-tile groups round-robin across the
in-flight chunks so the PE's in-order queue always has a ready matmul (the
PE p-state throttle drops the clock to 1.2 GHz after an idle gap).  T2 is
issued at the head of each DVE step because it gates the inv matmuls.
Evictions are split between the Act engine and DVE ('aah' schedule); the
host folds sign_flip, pre-permutes x into the interleaved polyphase order,
un-permutes y, and adds bias.

Data-parallel over rows: 16384 rows split 8 ways; constants replicated.
"""
from contextlib import ExitStack

import numpy as np
import ml_dtypes

import concourse.mybir as mybir
import concourse.bacc as bacc
import concourse.tile as tile
from concourse.bass_utils import run_bass_kernel_spmd

N_CORES = 8
ROWS = 16384
RPC = ROWS // N_CORES      # 2048 rows per core
F = 4096
P = 512                    # circulant block size
Q = 4                      # polyphase factor
PP = P // Q                # 128 transform length
NB = 8                     # circulant blocks per side
NC = NB * Q                # 32 polyphase components per side
CK = 256                   # rows per pipelined chunk
FREE = CK * NC             # 8192 free elems per chunk
_NC_CACHE = {}

DT = mybir.dt.bfloat16
DTO = mybir.dt.float32
ACT_COPY = mybir.ActivationFunctionType.Copy


def build_transforms_g(lam_r, lam_i, PP, dtype=np.float64):
    """Real-folded DFT transforms for circulants of size PP.
    lam_*: (CO, CI, PP) spectra. Returns Fe [c,s], Fi [t,c], M [CO,CI,c,c]."""
    s = np.arange(PP)
    f = np.arange(1, PP // 2)
    theta = 2 * np.pi * np.outer(f, s) / PP

    Fe = np.zeros((PP, PP), dtype)
    Fe[0, :] = 1.0
    Fe[1, :] = (-1.0) ** s
    Fe[2::2, :] = np.cos(theta)
    Fe[3::2, :] = -np.sin(theta)

    Fi = np.zeros((PP, PP), dtype)
    Fi[:, 0] = 1.0 / PP
    Fi[:, 1] = ((-1.0) ** s) / PP
    Fi[:, 2::2] = np.cos(theta).T / PP
    Fi[:, 3::2] = -np.sin(theta).T / PP

    CO, CI = lam_r.shape[:2]
    M = np.zeros((CO, CI, PP, PP), dtype)
    M[:, :, 0, 0] = lam_r[:, :, 0]
    M[:, :, 1, 1] = lam_r[:, :, PP // 2]
    l1r = lam_r[:, :, 1:PP // 2]; l1i = lam_i[:, :, 1:PP // 2]
    l2r = lam_r[:, :, :PP // 2:-1]; l2i = lam_i[:, :, :PP // 2:-1]
    ce = np.arange(2, PP, 2); co = ce + 1
    M[:, :, ce, ce] = l1r + l2r
    M[:, :, ce, co] = l2i - l1i
    M[:, :, co, ce] = l1i - l2i
    M[:, :, co, co] = l1r + l2r
    return Fe, Fi, M


def poly_spectra(spectral_real, spectral_imag):
    """Lam512 (8,8,512) -> polyphase 128-spectra lam[(4o+r),(4i+q),f]."""
    Lam = spectral_real.astype(np.complex128) + 1j * spectral_imag.astype(np.complex128)
    w = np.fft.ifft(Lam, axis=-1).real                    # (8,8,512) symbols
    m = np.arange(PP)
    r = np.arange(Q)[:, None, None]
    q = np.arange(Q)[None, :, None]
    idx = (Q * m[None, None, :] + r - q) % P              # (r, q, m)
    sym = w[:, :, idx]                                    # (o, i, r, q, m)
    lam = np.fft.fft(sym, axis=-1)                        # (o, i, r, q, f)
    lam = lam.transpose(0, 2, 1, 3, 4).reshape(NC, NC, PP)
    return lam


def host_transforms(spectral_real, spectral_imag):
    lam = poly_spectra(spectral_real, spectral_imag)
    Fe, Fi, M = build_transforms_g(lam.real, lam.imag, PP)
    # SBUF partition order p = 32j+g <-> natural coord 4g+j (so the corner
    # turn STs walk 32x32 blocks); PI maps p -> coord.
    PI = np.array([4 * g + j for j in range(Q) for g in range(NC)])
    fwdT = np.ascontiguousarray(Fe[PI, :].T)              # lhsT [s, p]
    invT = np.ascontiguousarray(Fi[:, PI].T)              # lhsT [p, t]
    # mid lhsT per coord group g: [k=(32j+c), m=(32j'+c')]
    midT = np.empty((NC, PP, PP))
    for g in range(NC):
        blk = M[:, :, 4 * g:4 * g + 4, 4 * g:4 * g + 4]   # (c', c, j', j)
        midT[g] = blk.transpose(3, 1, 2, 0).reshape(PP, PP)
    return fwdT, invT, midT


def _bf16(a):
    return np.asarray(a, np.float32).astype(ml_dtypes.bfloat16)


def build_nc(repeat: int = 1, rpc: int = RPC, variant: str = "g1",
             esched: str = "aah", tgrain: int = 2048,
             pcfg: str = "f1024x2,m512x2,i512x2"):
    key = (rpc, repeat, variant, esched, tgrain, pcfg)
    if key in _NC_CACHE:
        return _NC_CACHE[key]
    nc = bacc.Bacc("TRN2", target_bir_lowering=False, debug=False,
                   num_devices=N_CORES)
    n_chunks = rpc // CK
    xT = nc.dram_tensor("xT", [n_chunks, PP, FREE], DT, kind="ExternalInput")
    fwdT = nc.dram_tensor("fwdT", [PP, PP], DT, kind="ExternalInput")
    invT = nc.dram_tensor("invT", [PP, PP], DT, kind="ExternalInput")
    midT = nc.dram_tensor("midT", [NC, PP, PP], DT, kind="ExternalInput")
    yT = nc.dram_tensor("yT", [n_chunks, PP, FREE], DT, kind="ExternalOutput")

    with tile.TileContext(nc) as tc:
        with ExitStack() as ctx:
            const = ctx.enter_context(tc.tile_pool(name="const", bufs=1))
            fwd_sb = const.tile([PP, PP], DT)
            inv_sb = const.tile([PP, PP], DT)
            mid_sb = const.tile([PP, NC * PP], DT)
            nc.sync.dma_start(fwd_sb[:], fwdT[:, :])
            nc.sync.dma_start(inv_sb[:], invT[:, :])
            nc.sync.dma_start(
                mid_sb[:].rearrange("p (g c) -> p g c", g=NC),
                midT[:].rearrange("g p c -> p g c"))

            Ipool = ctx.enter_context(tc.tile_pool(name="I", bufs=2))
            Spool = ctx.enter_context(tc.tile_pool(name="S", bufs=2))
            Wpool = ctx.enter_context(tc.tile_pool(name="W", bufs=2))
            Zpool = ctx.enter_context(tc.tile_pool(name="Z", bufs=2))
            Vpool = ctx.enter_context(tc.tile_pool(name="V", bufs=2))
            Ypool = ctx.enter_context(tc.tile_pool(name="Y", bufs=2))
            pspec = {}
            for part in pcfg.split(","):
                size, bufs = part[1:].split("x")
                pspec[part[0]] = (int(size), int(bufs))
            szf, bf = pspec["f"]; szm, bm = pspec["m"]; szi, bi = pspec["i"]
            psf = ctx.enter_context(tc.tile_pool(name="psf", bufs=bf,
                                                 space="PSUM"))
            psm = ctx.enter_context(tc.tile_pool(name="psm", bufs=bm,
                                                 space="PSUM"))
            psi = ctx.enter_context(tc.tile_pool(name="psi", bufs=bi,
                                                 space="PSUM"))

            ecnt = [0]

            def evict(site, dst, src):
                e = esched[site]
                if e == "h":
                    e = "ad"[ecnt[0] % 2]
                    ecnt[0] += 1
                if e == "a":
                    nc.scalar.activation(dst, src, ACT_COPY)
                else:
                    nc.vector.tensor_copy(out=dst, in_=src)

            live = {}
            NT = tgrain

            def step(t):
                c0 = t                       # dma-in
                c1 = t - 1                   # fwd + E1 + T1
                c2 = t - 2                   # mid + E2
                c3 = t - 3                   # T2 + inv + E3 + dma-out
                if 0 <= c0 < n_chunks:
                    Xin = Ipool.tile([PP, FREE], DT, tag="I", name="It")
                    nc.sync.dma_start(Xin[:], xT[c0])
                    live[c0] = {"Xin": Xin}
                if 0 <= c3 < n_chunks:
                    st3 = live[c3]
                    V = Vpool.tile([PP, FREE], DT, tag="V", name="Vt")
                    nc.vector.transpose(V[:], st3["Z"][:])
                    st3["V"] = V
                    st3["Yb"] = Ypool.tile([PP, FREE], DT, tag="Y", name="Yt")
                if 0 <= c1 < n_chunks:
                    st1 = live[c1]
                    st1["W"] = Wpool.tile([PP, FREE], DT, tag="W", name="Wt")
                if 0 <= c2 < n_chunks:
                    st2 = live[c2]
                    st2["Z"] = Zpool.tile([PP, FREE], DT, tag="Z", name="Zt")
                    st2["Wv"] = st2["W"][:].rearrange("p (r g) -> p g r", g=NC)
                    st2["Zv"] = st2["Z"][:].rearrange("p (r g) -> p r g", g=NC)

                nm = FREE // szm          # mid tiles per chunk
                ni = FREE // szi          # inv tiles per chunk
                nf = FREE // szf          # fwd tiles per chunk
                gpm = szm // CK           # g's per mid tile
                for r in range(16):
                    if 0 <= c2 < n_chunks and r % (16 // nm) == 0:
                        st = live[c2]
                        m = r // (16 // nm)
                        ps = psm.tile([PP, szm], DTO, tag="m", name="mps")
                        for k in range(gpm):
                            g = m * gpm + k
                            nc.tensor.matmul(
                                ps[:, k * CK:(k + 1) * CK],
                                mid_sb[:, g * PP:(g + 1) * PP],
                                st["Wv"][:, g, :], start=True, stop=True)
                        evict(1, st["Zv"][:, :, m * gpm:(m + 1) * gpm],
                              ps[:].rearrange("p (g r) -> p r g", g=gpm))
                    if 0 <= c3 < n_chunks and r % (16 // ni) == 0:
                        st = live[c3]
                        i = r // (16 // ni)
                        ps = psi.tile([PP, szi], DTO, tag="i", name="ips")
                        for u in range(szi // 512):
                            v0 = i * szi + u * 512
                            nc.tensor.matmul(ps[:, u * 512:(u + 1) * 512],
                                             inv_sb[:], st["V"][:, v0:v0 + 512],
                                             start=True, stop=True)
                        evict(2, st["Yb"][:, i * szi:(i + 1) * szi], ps[:])
                    if 0 <= c1 < n_chunks and r % (16 // nf) == 0:
                        st = live[c1]
                        h = r // (16 // nf)
                        if (h * szf) % NT == 0:
                            st["Xs"] = Spool.tile([PP, NT], DT, tag="S",
                                                  name="St")
                        ps = psf.tile([PP, szf], DTO, tag="f", name="fps")
                        for u in range(szf // 512):
                            k0 = h * szf + u * 512
                            nc.tensor.matmul(ps[:, u * 512:(u + 1) * 512],
                                             fwd_sb[:],
                                             st["Xin"][:, k0:k0 + 512],
                                             start=True, stop=True)
                        ho = (h * szf) % NT
                        evict(0, st["Xs"][:, ho:ho + szf], ps[:])
                        if ho + szf == NT:
                            t0 = h * szf + szf - NT
                            nc.vector.transpose(
                                st["W"][:, t0:t0 + NT], st["Xs"][:])
                if 0 <= c3 < n_chunks:
                    st3 = live.pop(c3)
                    nc.sync.dma_start(yT[c3], st3["Yb"][:])

            def body(_=None):
                for t in range(n_chunks + 3):
                    step(t)

            if repeat == 1:
                body()
            else:
                with tc.For_i(0, repeat, 1) as it:
                    body(it)
    nc.compile()
    _NC_CACHE[key] = nc
    return nc


def make_in_maps(x, spectral_real, spectral_imag, sign_flip):
    fwdT, invT, midT = host_transforms(spectral_real, spectral_imag)
    fwdT = _bf16(fwdT); invT = _bf16(invT); midT = _bf16(midT)
    xs = (x.reshape(-1, F) * sign_flip[None, :].astype(np.float32))
    # X0[m, 32*row + 4i+q] = xs[row, 512i + 4m + q]
    t = xs.reshape(ROWS, NB, PP, Q)                   # [row, i, m, q]
    xp = _bf16(t.transpose(2, 0, 1, 3).reshape(PP, ROWS, NC))
    n_chunks = RPC // CK
    in_maps = []
    for c in range(N_CORES):
        shard = xp[:, c * RPC:(c + 1) * RPC, :]       # [128, RPC, 32]
        xh = shard.reshape(PP, n_chunks, FREE).transpose(1, 0, 2)
        in_maps.append({
            "xT": np.ascontiguousarray(xh),
            "fwdT": fwdT, "invT": invT, "midT": midT,
        })
    return in_maps


def kernel(x, spectral_real, spectral_imag, sign_flip, bias):
    x = np.asarray(x, np.float32)
    spectral_real = np.asarray(spectral_real, np.float32)
    spectral_imag = np.asarray(spectral_imag, np.float32)
    sign_flip = np.asarray(sign_flip, np.float32)
    bias = np.asarray(bias, np.float32)
    batch_shape = x.shape[:-1]

    in_maps = make_in_maps(x, spectral_real, spectral_imag, sign_flip)
    nc = build_nc()
    res = run_bass_kernel_spmd(nc, in_maps, list(range(N_CORES)))
    # yT[ch, t, 32*row + 4o+r2] -> y[ch*CK+row, 512o + 4t + r2]
    n_chunks = RPC // CK
    parts = []
    for c in range(N_CORES):
        yh = np.asarray(res.results[c]["yT"], np.float32)   # [nch, 128, FREE]
        yp = yh.reshape(n_chunks, PP, CK, NB, Q)            # [ch, t, row, o, r2]
        yp = yp.transpose(0, 2, 3, 1, 4).reshape(RPC, F)
        parts.append(yp)
    y = np.concatenate(parts, axis=0)
    y = y + bias[None, :]
    return y.reshape(*batch_shape, F).astype(np.float32)
